# revision 18
# baseline (speedup 1.0000x reference)
"""ChamferLoss kernel for 8 Trainium2 NeuronCores (Bass/Tile).

Problem (B=4, N=M=8192):
  d2[b,n,m] = |P[b,n] - R[b,m]|^2  (expanded form)
  total = mean(min_m d2) + mean(min_n d2)            (chamfer, both directions)
        + mean(|ref_sdfs[b, argmin_n d2[b,:,m]] - predicted_sdfs[b,m]|)
        + mean(|ref_colors[b, argmin_n d2[b,:,m]] - predicted_colors[b,m]|)

Sharding: core c handles batch b=c//2 and m-half h=c%2 (all n).

Device per core (orientation: n on partitions, m on free):
  Phase A  - K=16 bf16 hi/lo-split matmuls (4x row-packed via tile_position)
             reproduce fp32-grade d2 tiles [128n x 512m] in PSUM at bf16
             streaming rate; ACT relu-casts to a per-strip S tile
             [128, 32768] bf16 kept in SBUF; DVE TT-min accumulates colmin
             (acc2) and a per-strip free-dim reduce gives rowmin partials.
  Tail(j)  - fold acc2 to [128,512]; GPSIMD partition-reduce -> colmin[m]
             (m-ordered); broadcast via K=1 ones-matmul.
  Phase B  - eq-mask of the STORED S vs broadcast colmin on GPSIMD (no
             d2 recompute), iota-weight matmuls accumulate
             (count, sum_p, sum_i) per m.
Host: combines mins (means), decodes argmin idx=128*i+p, fixes multi-candidate
      buckets by exact recompute, gathers sdf/colors, final scalar in f64.
"""
import os
import numpy as np
import ml_dtypes
from contextlib import ExitStack

import concourse.bass as bass
import concourse.bacc as bacc
import concourse.tile as tile
from concourse import mybir, bass_isa
from concourse.bass_utils import run_bass_kernel_spmd

B, N, M = 4, 8192, 8192
NCORES = 8
MH = M // 2            # m-range per core
K = 16                 # hi/lo augmented contraction dim
PB = 128               # partition block (n per block)
NBLK = N // PB         # 64 n-blocks
STRIP = 512            # m per strip (PSUM bank)
NSTRIP = MH // STRIP   # 8 strips per core
GRP = 4                # row-packed group size (n-blocks per PE burst)
NGRP = NBLK // GRP     # 16 groups
GW = GRP * STRIP       # 2048: group width in S
SW = NGRP * GW         # 32768: strip width in S
MCH = 2                # groups per phase-B mask chunk
CHW = MCH * GW         # 4096: mask chunk width
NCH = NGRP // MCH      # 8 mask chunks per strip

F32 = mybir.dt.float32
BF16 = mybir.dt.bfloat16
BIG = 3.0e38

_CACHE = {}


def _build_bass():
    nc = bacc.Bacc("TRN2", target_bir_lowering=False, debug=False,
                   num_devices=NCORES)
    ins = {
        "aug": nc.dram_tensor("aug", [K, N + MH], BF16, kind="ExternalInput").ap(),
        "iotaW": nc.dram_tensor("iotaW", [PB, NBLK * 3], BF16,
                                kind="ExternalInput").ap(),
    }
    outs = {
        "rowmin_o": nc.dram_tensor("rowmin_o", [PB, NBLK], F32,
                                   kind="ExternalOutput").ap(),
        "colmin_o": nc.dram_tensor("colmin_o", [NSTRIP, STRIP], F32,
                                   kind="ExternalOutput").ap(),
        "iota_o": nc.dram_tensor("iota_o", [NSTRIP, PB, STRIP], F32,
                                 kind="ExternalOutput").ap(),
    }
    with tile.TileContext(nc) as tc:
        _kernel_body(tc, outs, ins)
    nc.compile()
    return nc


def _kernel_body(tc, outs, ins):
    nc = tc.nc
    with ExitStack() as ctx:
        const = ctx.enter_context(tc.tile_pool(name="const", bufs=1))
        spool = ctx.enter_context(tc.tile_pool(name="spool", bufs=1))
        a2pool = ctx.enter_context(tc.tile_pool(name="a2pool", bufs=2))
        crep = ctx.enter_context(tc.tile_pool(name="crep", bufs=2))
        mpool = ctx.enter_context(tc.tile_pool(name="mpool", bufs=2))
        small = ctx.enter_context(tc.tile_pool(name="small", bufs=2))
        iosb_pool = ctx.enter_context(tc.tile_pool(name="iosb", bufs=2))
        pspool = ctx.enter_context(tc.tile_pool(name="ps", bufs=3, space="PSUM"))
        iops = ctx.enter_context(tc.tile_pool(name="iops", bufs=1, space="PSUM"))

        # ---- load inputs (aug matrix replicated at row-group offsets).
        # One DMA per replica (single input tensor) so each matmul has at most
        # one sync wait (LDW wait-slot limit); iotaW is staged through a DVE
        # copy so PE deps on it ride the DVE semaphore.
        aug_t = const.tile([128, N + MH], BF16, tag="aug")
        iotaW_st = const.tile([PB, NBLK * 3], BF16, tag="iotaW_st")
        iotaW_t = const.tile([PB, NBLK * 3], BF16, tag="iotaW")
        for g in range(GRP):
            nc.sync.dma_start(out=aug_t[32*g:32*g+K, :], in_=ins["aug"][:, :])
        nc.sync.dma_start(out=iotaW_st[:, :], in_=ins["iotaW"][:, :])
        nc.vector.tensor_copy(iotaW_t, iotaW_st)

        # ---- accumulators: acc1[gi] holds per-(p, g, m-in-strip) running
        # mins across strips (dir-1 / rowmin); initialized by copy at j=0.
        acc1 = [const.tile([128, GW], BF16, tag=f"acc1_{gi}",
                           name=f"acc1_{gi}")
                for gi in range(NGRP)]

        def emit_group_matmuls(j, gi):
            """4 row-packed K=16 bf16 matmuls -> two [128,1024] psum tiles."""
            ps0 = pspool.tile([128, 2 * STRIP], F32, tag="ps")
            ps1 = pspool.tile([128, 2 * STRIP], F32, tag="ps")
            pss = (ps0, ps0, ps1, ps1)
            for g in range(GRP):
                i = GRP * gi + g
                nc.tensor.matmul(
                    out=pss[g][:, (g % 2) * STRIP:(g % 2 + 1) * STRIP],
                    lhsT=aug_t[32*g:32*g+K, i*PB:(i+1)*PB],
                    rhs=aug_t[32*g:32*g+K, N + j*STRIP:N + (j+1)*STRIP],
                    start=True, stop=True, tile_position=(32*g, 0))
            return ps0, ps1

        def emit_phase_a(j):
            """Matmul + cast into the stored S(j); accumulate both mins."""
            s = spool.tile([128, SW], BF16, tag="S", name=f"S_{j}")
            acc2 = a2pool.tile([128, GW], BF16, tag="acc2", name=f"acc2_{j}")
            for gi in range(NGRP):
                ps0, ps1 = emit_group_matmuls(j, gi)
                sg = s[:, gi*GW:(gi+1)*GW]
                nc.scalar.activation(s[:, gi*GW:gi*GW + GW//2], ps0,
                                     mybir.ActivationFunctionType.Relu)
                nc.scalar.activation(s[:, gi*GW + GW//2:(gi+1)*GW], ps1,
                                     mybir.ActivationFunctionType.Relu)
                if gi == 0:
                    nc.vector.tensor_copy(acc2, sg)
                else:
                    nc.vector.tensor_tensor(acc2, acc2, sg, mybir.AluOpType.min)
                if j == 0:
                    nc.vector.tensor_copy(acc1[gi], sg)
                else:
                    nc.vector.tensor_tensor(acc1[gi], acc1[gi], sg,
                                            mybir.AluOpType.min)
            return s, acc2

        def emit_tail(j, acc2):
            """colmin[m] from acc2: fold, negate, then a cross-partition
            max all-reduce (min unsupported cross-lane) which also handles
            the broadcast to all 128 partitions."""
            nc.vector.tensor_tensor(acc2[:, 0:1024], acc2[:, 0:1024],
                                    acc2[:, 1024:2048], mybir.AluOpType.min)
            nc.vector.tensor_tensor(acc2[:, 0:512], acc2[:, 0:512],
                                    acc2[:, 512:1024], mybir.AluOpType.min)
            neg = small.tile([128, STRIP], BF16, tag="neg")
            nc.scalar.activation(neg, acc2[:, 0:512],
                                 mybir.ActivationFunctionType.Copy, scale=-1.0)
            nar = crep.tile([128, STRIP], F32, tag="nar")
            nc.gpsimd.partition_all_reduce(nar, neg, channels=128,
                                           reduce_op=bass_isa.ReduceOp.max)
            cm32 = small.tile([1, STRIP], F32, tag="cm32")
            nc.scalar.activation(cm32, nar[0:1, :],
                                 mybir.ActivationFunctionType.Copy, scale=-1.0)
            nc.sync.dma_start(out=outs["colmin_o"][j:j+1, :], in_=cm32[0:1, :])
            rep = crep.tile([128, CHW], BF16, tag="crep")
            nc.scalar.activation(rep[:, 0:512], nar,
                                 mybir.ActivationFunctionType.Copy, scale=-1.0)
            nc.scalar.activation(rep[:, 512:1024], rep[:, 0:512],
                                 mybir.ActivationFunctionType.Copy)
            nc.scalar.activation(rep[:, 1024:2048], rep[:, 0:1024],
                                 mybir.ActivationFunctionType.Copy)
            nc.scalar.activation(rep[:, 2048:4096], rep[:, 0:2048],
                                 mybir.ActivationFunctionType.Copy)
            return rep

        def emit_phase_b(j, s, rep):
            """eq-mask of stored S vs colmin (GPSIMD) + iota matmuls."""
            iot = iops.tile([128, STRIP], F32, tag="iops")
            for c in range(NCH // 2):
                msk = mpool.tile([128, 2 * CHW], BF16, tag="msk", bufs=1)
                for half in range(2):
                    nc.vector.tensor_tensor(
                        msk[:, half*CHW:(half+1)*CHW],
                        s[:, (2*c+half)*CHW:(2*c+half+1)*CHW],
                        rep, mybir.AluOpType.is_equal)
                for gg in range(2 * MCH * GRP):
                    i = c * 2 * MCH * GRP + gg
                    g = i % GRP
                    nc.tensor.matmul(
                        out=iot[32*g:32*g+3, :],
                        lhsT=iotaW_t[:, 3*i:3*i+3],
                        rhs=msk[:, gg*STRIP:(gg+1)*STRIP],
                        start=(i < GRP), stop=(i >= NBLK - GRP),
                        tile_position=(0, 32*g))
            io_sb = iosb_pool.tile([128, STRIP], F32, tag="iosb")
            nc.scalar.activation(io_sb, iot, mybir.ActivationFunctionType.Copy)
            nc.sync.dma_start(out=outs["iota_o"][j, :, :], in_=io_sb[:, :])

        # ---- main loop: S is single-buffered, so B(j) runs before A(j+1)
        # can overwrite it; engines still pipeline across the j boundary.
        for j in range(NSTRIP):
            s, acc2 = emit_phase_a(j)
            rep = emit_tail(j, acc2)
            emit_phase_b(j, s, rep)

        # ---- rowmin finalization: per-block min over the strip columns ----
        rowmin_sb = const.tile([128, NBLK], F32, tag="rowmin")
        for gi in range(NGRP):
            nc.vector.tensor_reduce(
                rowmin_sb[:, GRP*gi:GRP*(gi+1)],
                acc1[gi].rearrange("p (g c) -> p g c", c=STRIP),
                axis=mybir.AxisListType.X, op=mybir.AluOpType.min)
        nc.sync.dma_start(out=outs["rowmin_o"][:, :], in_=rowmin_sb[:, :])


def _host_prep(inputs):
    """Per-core input dicts."""
    pp = np.asarray(inputs["predicted_points"], dtype=np.float32)   # [B, N, 3]
    rp = np.asarray(inputs["ref_points"], dtype=np.float32)         # [B, M, 3]
    iota = np.zeros((PB, NBLK * 3), np.float32)
    for i in range(NBLK):
        iota[:, 3*i + 0] = 1.0
        iota[:, 3*i + 1] = np.arange(PB)
        iota[:, 3*i + 2] = float(i)
    iota = iota.astype(ml_dtypes.bfloat16)

    def hilo(x):
        hi = x.astype(ml_dtypes.bfloat16).astype(np.float32)
        lo = (x - hi).astype(ml_dtypes.bfloat16).astype(np.float32)
        return hi, lo

    in_maps = []
    for c in range(NCORES):
        b, h = c // 2, c % 2
        P = pp[b]                                   # [N, 3]
        R = rp[b, h*MH:(h+1)*MH]                    # [MH, 3]
        Ph, Pl = hilo(P)
        Rh, Rl = hilo(R)
        p2h, p2l = hilo((P.astype(np.float64) ** 2).sum(-1).astype(np.float32))
        r2h, r2l = hilo((R.astype(np.float64) ** 2).sum(-1).astype(np.float32))
        aug = np.zeros((K, N + MH), np.float32)
        for cc in range(3):
            aug[4*cc + 0, :N] = Ph[:, cc]
            aug[4*cc + 1, :N] = Ph[:, cc]
            aug[4*cc + 2, :N] = Pl[:, cc]
            aug[4*cc + 3, :N] = Pl[:, cc]
            aug[4*cc + 0, N:] = -2.0 * Rh[:, cc]
            aug[4*cc + 1, N:] = -2.0 * Rl[:, cc]
            aug[4*cc + 2, N:] = -2.0 * Rh[:, cc]
            aug[4*cc + 3, N:] = -2.0 * Rl[:, cc]
        aug[12, :N] = p2h
        aug[13, :N] = p2l
        aug[14, :N] = 1.0
        aug[15, :N] = 1.0
        aug[12, N:] = 1.0
        aug[13, N:] = 1.0
        aug[14, N:] = r2h
        aug[15, N:] = r2l
        in_maps.append({"aug": aug.astype(ml_dtypes.bfloat16),
                        "iotaW": np.ascontiguousarray(iota)})
    return in_maps


def _host_combine(inputs, results):
    pp = np.asarray(inputs["predicted_points"], dtype=np.float64)
    rp = np.asarray(inputs["ref_points"], dtype=np.float64)
    psdf = np.asarray(inputs["predicted_sdfs"], dtype=np.float64)     # [B, N]
    pcol = np.asarray(inputs["predicted_colors"], dtype=np.float64)   # [B, N, 3]
    rsdf = np.asarray(inputs["ref_sdfs"], dtype=np.float64)[..., 0]   # [B, M]
    rcol = np.asarray(inputs["ref_colors"], dtype=np.float64)         # [B, M, 3]

    rowmin = np.empty((B, N), np.float64)
    colmin = np.empty((B, M), np.float64)
    idx = np.empty((B, M), np.int64)
    bad = []

    for b in range(B):
        r0, r1 = results[2*b], results[2*b + 1]
        rm = np.minimum(r0["rowmin_o"], r1["rowmin_o"])       # [128, 64]
        rowmin[b] = rm.T.reshape(-1).astype(np.float64)       # n = 128*i + p
        for h, res in ((0, r0), (1, r1)):
            colmin[b, h*MH:(h+1)*MH] = res["colmin_o"].reshape(-1)
            io = res["iota_o"]                                 # [8, 128, 512]
            cnt = np.zeros((NSTRIP, STRIP)); sp = np.zeros((NSTRIP, STRIP))
            si = np.zeros((NSTRIP, STRIP))
            for c4 in range(GRP):
                cnt += io[:, 32*c4 + 0, :]
                sp += io[:, 32*c4 + 1, :]
                si += io[:, 32*c4 + 2, :]
            cnt = cnt.reshape(-1); sp = sp.reshape(-1); si = si.reshape(-1)
            mslice = slice(h*MH, (h+1)*MH)
            ii = np.rint(128.0 * si + sp).astype(np.int64)
            idx[b, mslice] = np.clip(ii, 0, N - 1)
            badm = np.nonzero(np.rint(cnt).astype(np.int64) != 1)[0]
            for ml in badm:
                bad.append((b, h*MH + int(ml)))

    # fixups for multi-candidate buckets: recompute the column mimicking the
    # reference's f32 expanded form (ties resolve to first occurrence, like
    # jnp.argmin); f32 keeps the tie structure maximally close to the ref.
    if bad:
        pp32 = pp.astype(np.float32)
        rp32 = rp.astype(np.float32)
        np2 = {bb: (pp32[bb] * pp32[bb]).sum(-1) for bb in set(b for b, _ in bad)}
        for (b, m) in bad:
            r = rp32[b, m]
            g = pp32[b] @ r                                   # f32 dot [N]
            rr2 = np.float32((r * r).sum())
            dcol = np.maximum(np2[b] + rr2 - np.float32(2.0) * g, np.float32(0.0))
            idx[b, m] = int(np.argmin(dcol))
            colmin[b, m] = float(dcol.min())

    cham = rowmin.mean() + colmin.mean()
    g_sdf = np.take_along_axis(rsdf, idx, axis=1)             # [B, M]
    sdf_l1 = np.abs(g_sdf - psdf).mean()
    g_col = np.take_along_axis(rcol, idx[..., None], axis=1)  # [B, M, 3]
    col_l1 = np.abs(g_col - pcol).mean()
    total = sdf_l1 + col_l1 + cham
    return np.float32(total)


def _next_tmpdir():
    base = os.environ.get("CHAMFER_TMPDIR")
    if not base:
        return None
    n = _CACHE["ncalls"] = _CACHE.get("ncalls", 0) + 1
    d = os.path.join(base, f"call{n}")
    os.makedirs(d, exist_ok=True)
    return d


def kernel(**inputs) -> np.ndarray:
    if "nc" not in _CACHE:
        _CACHE["nc"] = _build_bass()
    nc = _CACHE["nc"]
    in_maps = _host_prep(inputs)
    res = run_bass_kernel_spmd(
        nc, in_maps, core_ids=list(range(NCORES)),
        trace=bool(int(os.environ.get("CHAMFER_TRACE", "0"))),
        tmpdir=_next_tmpdir(),
    )
    if res.exec_time_ns is not None:
        _CACHE["exec_time_ns"] = res.exec_time_ns
        _CACHE["last_results"] = res
    return _host_combine(inputs, res.results)
